# revision 1
# baseline (speedup 1.0000x reference)
# RBF Gram matrix kernel for Trainium2 (8 NeuronCores, SPMD).
#
# reference:  G[i, j] = exp(-gamma * ||x_i - y_j||^2)
#                    = exp(2*gamma*((x@y^T)[i,j] - 0.5*||y_j||^2) - gamma*||x_i||^2)
#
# Sharding: row-shard x across 8 cores (1024 rows each), replicate y.
# Each core computes a [1024, 8192] slice of G:
#   PE   : xy = x_c @ y^T       (bf16 in, fp32 PSUM, K=512 as 4 k-tiles)
#   DVE  : s  = xy + (-0.5*||y||^2)  (free-dim broadcast row, one wide op)
#   ACT  : o  = Exp(2*gamma*s + (-gamma*||x||^2))  (per-partition bias)
#   DMA  : o tile (bf16) -> DRAM; host upcasts to fp32
#
# x^T / y^T are shipped pre-permuted into the exact SBUF images so every
# prefetch chunk is one contiguous DMA.
import os

import numpy as np
import ml_dtypes

N_CORES = 8
N_FULL = 8192          # rows of x (and of G)
M_FULL = 8192          # rows of y (cols of G)
D = 512                # feature dim (contraction)
MC = N_FULL // N_CORES # 1024 rows of x per core
P = 128                # SBUF partitions
NT = 512               # moving-dim tile (max; one fp32 psum bank)
KT = D // P            # 4 k-tiles
MT = MC // P           # 8 m-tiles per core
NG = 1024              # psum group width: 2 banks
NGR = M_FULL // NG     # 8 n-groups

_cache = {}


def _build_program(scale2g: float, mc: int, n_full: int, d: int):
    """Build + compile the SPMD bass program. scale2g = 2*gamma immediate."""
    import concourse.mybir as mybir
    import concourse.tile as tile
    from concourse import bacc

    mt = mc // P
    kt = d // P
    ng_w = min(NG, n_full)
    ngroups = n_full // ng_w
    nnsub = ng_w // NT

    nc = bacc.Bacc("TRN2", target_bir_lowering=False, debug=False,
                   num_devices=N_CORES)

    # exact SBUF images (see kernel() for the host-side permutes)
    xT_d = nc.dram_tensor("xTb", [P, mt * kt * P], mybir.dt.bfloat16,
                          kind="ExternalInput").ap()
    yT_d = nc.dram_tensor("yTb", [P, ngroups * kt * ng_w], mybir.dt.bfloat16,
                          kind="ExternalInput").ap()
    y2_d = nc.dram_tensor("y2n", [1, n_full], mybir.dt.float32,
                          kind="ExternalInput").ap()
    x2_d = nc.dram_tensor("x2b", [P, mt], mybir.dt.float32,
                          kind="ExternalInput").ap()
    out_d = nc.dram_tensor("out", [mc, n_full], mybir.dt.bfloat16,
                           kind="ExternalOutput").ap()

    with tile.TileContext(nc) as tc:
        with (
            tc.tile_pool(name="resident", bufs=1) as res_pool,
            tc.tile_pool(name="psum", bufs=4, space="PSUM") as psum_pool,
            tc.tile_pool(name="sq", bufs=4) as s_pool,
            tc.tile_pool(name="ot", bufs=6) as o_pool,
        ):
            xT_sb = res_pool.tile([P, mt * kt * P], mybir.dt.bfloat16, tag="xT")
            yT_sb = res_pool.tile([P, ngroups * kt * ng_w], mybir.dt.bfloat16,
                                  tag="yT")
            y2r_sb = res_pool.tile([1, n_full], mybir.dt.float32, tag="y2r")
            y2_sb = res_pool.tile([P, n_full], mybir.dt.float32, tag="y2")
            x2_sb = res_pool.tile([P, mt], mybir.dt.float32, tag="x2")
            scr_sb = res_pool.tile([P, 2 * P], mybir.dt.bfloat16, tag="scr")

            def lhsT(k, m):
                c0 = (m * kt + k) * P
                return xT_sb[:, c0:c0 + P]

            def rhs(k, ng, nn):
                c0 = ((ng * nnsub + nn) * kt + k) * NT
                return yT_sb[:, c0:c0 + NT]

            # PE warm-up: short matmuls on zeroed scratch keep the HAM
            # activity window busy while the startup DMAs land, so the
            # real matmuls run at 2.4 GHz from the first one. The psum
            # slot is recycled by the pool afterwards.
            nc.vector.memset(scr_sb, 0.0)
            wps = psum_pool.tile([P, ng_w], mybir.dt.float32,
                                 name="wps", tag="ps")
            for _ in range(24):
                nc.tensor.matmul(wps[:, 0:P], lhsT=scr_sb[:, P:2 * P],
                                 rhs=scr_sb[:, 0:P], start=True, stop=True)

            def bcast_y2(ch):
                sl = slice(ch * ng_w, (ch + 1) * ng_w)
                nc.gpsimd.partition_broadcast(y2_sb[:, sl], y2r_sb[0:1, sl])

            def dma_yt_block(b):
                c0 = b * kt * NT
                nc.sync.dma_start(out=yT_sb[:, c0:c0 + kt * NT],
                                  in_=yT_d[:, c0:c0 + kt * NT])

            def dma_yt_chunk(ch):
                for bb in range(nnsub):
                    dma_yt_block(ch * nnsub + bb)

            # startup set, in critical-path order. The first y^T block is
            # split per k so its four 128KB pieces ride parallel DMA queues.
            nc.sync.dma_start(out=xT_sb[:, 0:kt * P], in_=xT_d[:, 0:kt * P])
            for k in range(kt):
                c0 = k * NT
                nc.sync.dma_start(out=yT_sb[:, c0:c0 + NT],
                                  in_=yT_d[:, c0:c0 + NT])
            nc.sync.dma_start(out=y2r_sb, in_=y2_d)
            nc.sync.dma_start(out=x2_sb, in_=x2_d)
            for bb in range(1, nnsub):
                dma_yt_block(bb)
            bcast_y2(0)
            if mt > 1:
                nc.sync.dma_start(out=xT_sb[:, kt * P:2 * kt * P],
                                  in_=xT_d[:, kt * P:2 * kt * P])
            if mt > 2:
                nc.sync.dma_start(out=xT_sb[:, 2 * kt * P:],
                                  in_=xT_d[:, 2 * kt * P:])
            if ngroups > 1:
                dma_yt_chunk(1)
                bcast_y2(1)

            for ng in range(ngroups):
                gsl = slice(ng * ng_w, (ng + 1) * ng_w)
                if ng + 2 < ngroups:
                    dma_yt_chunk(ng + 2)   # just-in-time prefetch
                    bcast_y2(ng + 2)
                for m in range(mt):
                    msl = slice(m * P, (m + 1) * P)
                    ps = psum_pool.tile([P, ng_w], mybir.dt.float32, tag="ps")
                    for k in range(kt):
                        for nn in range(nnsub):
                            nc.tensor.matmul(
                                ps[:, nn * NT:(nn + 1) * NT],
                                lhsT=lhsT(k, m),
                                rhs=rhs(k, ng, nn),
                                start=(k == 0),
                                stop=(k == kt - 1),
                            )
                    last = (ng == ngroups - 1) and (m == mt - 1)
                    if not last:
                        s = s_pool.tile([P, ng_w], mybir.dt.float32)
                        nc.vector.tensor_add(s, ps, y2_sb[:, gsl])
                        o = o_pool.tile([P, ng_w], mybir.dt.bfloat16)
                        nc.scalar.activation(
                            o, s, mybir.ActivationFunctionType.Exp,
                            bias=x2_sb[:, m:m + 1], scale=float(scale2g),
                        )
                        nc.sync.dma_start(out=out_d[msl, gsl], in_=o)
                    else:
                        # split the final drain chain to shorten the tail
                        for nn in range(nnsub):
                            nsl = slice(ng * ng_w + nn * NT,
                                        ng * ng_w + (nn + 1) * NT)
                            psl = slice(nn * NT, (nn + 1) * NT)
                            s = s_pool.tile([P, NT], mybir.dt.float32,
                                            name=f"sl{nn}", tag=f"sl{nn}")
                            nc.vector.tensor_add(s, ps[:, psl], y2_sb[:, nsl])
                            o = o_pool.tile([P, NT], mybir.dt.bfloat16,
                                            name=f"ol{nn}", tag=f"ol{nn}")
                            nc.scalar.activation(
                                o, s, mybir.ActivationFunctionType.Exp,
                                bias=x2_sb[:, m:m + 1], scale=float(scale2g),
                            )
                            nc.sync.dma_start(out=out_d[msl, nsl], in_=o)

    nc.compile()
    return nc


def _build_program_raw(scale2g: float, mc: int, n_full: int, d: int):
    """Raw-Bass build: explicit per-engine programs + hand-rolled semaphores.
    Avoids the Tile scheduler's ~7us prologue and ~10us exit butterfly."""
    from contextlib import ExitStack, contextmanager

    import concourse.bass as bass
    import concourse.mybir as mybir
    from concourse import bacc

    class _NoBarrierBlock(bass.BassBlock):
        """BassBlock whose exit emits per-engine drains but no all-engine
        barrier; cross-engine ordering is fully covered by our semaphores."""

        def __exit__(self, exc_type, exc_val, exc_tb):
            if exc_type is not None:
                return
            for engine, last_body in self.last_body.items():
                with self.bass.body(last_body, parent=self.bass.cur_bb,
                                    allow_existing_parent=True):
                    engine.br(self.end_bb)
            self.bass.switch_bb(self.end_bb)
            gpsimd_type = self.bass.gpsimd.engine
            for eng_type, eng in self.bass.engines.items():
                if eng_type == gpsimd_type:
                    continue
                dr = mybir.InstDrain(
                    name=self.bass.get_next_instruction_name(),
                    ins=[], outs=[], bass_is_fusable=False)
                dr.engine = eng_type
                eng.add_instruction(dr)

    @contextmanager
    def _no_barrier_block(nc):
        assert nc.cur_block is None
        blk = _NoBarrierBlock(nc, f"block_{nc.next_id()}")
        nc.cur_block = blk
        try:
            with blk:
                yield blk
        finally:
            nc.cur_block = None

    mt = mc // P
    kt = d // P
    ng_w = min(NG, n_full)
    ngroups = n_full // ng_w
    nnsub = ng_w // NT
    G = ngroups * mt
    S_SLOTS = 4            # psum slots (8 banks / 2)
    SS_SLOTS = 4           # SBUF s-staging slots (same ring as psum)
    O_SLOTS = 8            # output staging slots
    NWARM = 24

    nc = bacc.Bacc("TRN2", target_bir_lowering=False, debug=False,
                   num_devices=N_CORES)

    xT_d = nc.dram_tensor("xTb", [P, mt * kt * P], mybir.dt.bfloat16,
                          kind="ExternalInput").ap()
    yT_d = nc.dram_tensor("yTb", [P, ngroups * kt * ng_w], mybir.dt.bfloat16,
                          kind="ExternalInput").ap()
    y2_d = nc.dram_tensor("y2n", [P, n_full], mybir.dt.float32,
                          kind="ExternalInput").ap()
    x2_d = nc.dram_tensor("x2b", [P, mt], mybir.dt.float32,
                          kind="ExternalInput").ap()
    out_d = nc.dram_tensor("out", [mc, n_full], mybir.dt.bfloat16,
                           kind="ExternalOutput").ap()

    with ExitStack() as ctx:
        ec = ctx.enter_context
        xT_sb = ec(nc.sbuf_tensor([P, mt * kt * P], mybir.dt.bfloat16))
        yT_sb = ec(nc.sbuf_tensor([P, ngroups * kt * ng_w], mybir.dt.bfloat16))
        y2_sb = ec(nc.sbuf_tensor([P, n_full], mybir.dt.float32))
        x2_sb = ec(nc.sbuf_tensor([P, mt], mybir.dt.float32))
        scr_sb = ec(nc.sbuf_tensor([P, 2 * P], mybir.dt.bfloat16))
        s_sb = ec(nc.sbuf_tensor([P, SS_SLOTS * ng_w], mybir.dt.float32))
        o_sb = ec(nc.sbuf_tensor([P, O_SLOTS * ng_w], mybir.dt.bfloat16))
        ps = ec(nc.psum_tensor([P, S_SLOTS * ng_w], mybir.dt.float32))

        s_scr = ec(nc.semaphore(name="s_scr"))
        s_xT = [ec(nc.semaphore(name=f"s_xT{i}")) for i in range(3)]
        s_yb = [ec(nc.semaphore(name=f"s_yb{i}"))
                for i in range(ngroups * nnsub)]
        yb_cnt = [0] * (ngroups * nnsub)
        s_y2c = [ec(nc.semaphore(name=f"s_y2c{i}")) for i in range(ngroups)]
        s_x2 = ec(nc.semaphore(name="s_x2"))
        s_mm = ec(nc.semaphore(name="s_mm"))
        s_dve = ec(nc.semaphore(name="s_dve"))
        s_act = ec(nc.semaphore(name="s_act"))
        s_osl = [ec(nc.semaphore(name=f"s_osl{i}")) for i in range(O_SLOTS)]

        def lhsT(k, m):
            c0 = (m * kt + k) * P
            return xT_sb[:, c0:c0 + P]

        def rhs(k, ng, nn):
            c0 = ((ng * nnsub + nn) * kt + k) * NT
            return yT_sb[:, c0:c0 + NT]

        with _no_barrier_block(nc) as block:

            def dma_chunk(sync, ci, split_first=False):
                for bb in range(nnsub):
                    b = ci * nnsub + bb
                    b0 = b * kt * NT
                    if split_first:
                        for k in range(kt):
                            sync.dma_start(
                                out=yT_sb[:, b0 + k * NT:b0 + (k + 1) * NT],
                                in_=yT_d[:, b0 + k * NT:b0 + (k + 1) * NT]
                            ).then_inc(s_yb[b], 16)
                            yb_cnt[b] += 16
                    else:
                        sync.dma_start(out=yT_sb[:, b0:b0 + kt * NT],
                                       in_=yT_d[:, b0:b0 + kt * NT]
                                       ).then_inc(s_yb[b], 16)
                        yb_cnt[b] += 16
                g0 = ci * ng_w
                sync.dma_start(out=y2_sb[:, g0:g0 + ng_w],
                               in_=y2_d[:, g0:g0 + ng_w]
                               ).then_inc(s_y2c[ci], 16)

            @block.sync
            def _(sync):
                sync.dma_start(out=xT_sb[:, 0:kt * P],
                               in_=xT_d[:, 0:kt * P]).then_inc(s_xT[0], 16)
                dma_chunk(sync, 0, split_first=True)
                sync.dma_start(out=x2_sb[:], in_=x2_d).then_inc(s_x2, 16)
                if mt > 1:
                    sync.dma_start(out=xT_sb[:, kt * P:2 * kt * P],
                                   in_=xT_d[:, kt * P:2 * kt * P]
                                   ).then_inc(s_xT[1], 16)
                if mt > 2:
                    sync.dma_start(out=xT_sb[:, 2 * kt * P:],
                                   in_=xT_d[:, 2 * kt * P:]).then_inc(s_xT[2], 16)
                if ngroups > 1:
                    dma_chunk(sync, 1)
                for ng in range(ngroups):
                    if ng + 2 < ngroups:
                        dma_chunk(sync, ng + 2)
                    gsl = slice(ng * ng_w, (ng + 1) * ng_w)
                    for m in range(mt):
                        g = ng * mt + m
                        sl = g % O_SLOTS
                        msl = slice(m * P, (m + 1) * P)
                        if g < G - 1:
                            sync.wait_ge(s_act, g + 1)
                            sync.dma_start(
                                out=out_d[msl, gsl],
                                in_=o_sb[:, sl * ng_w:(sl + 1) * ng_w]
                            ).then_inc(s_osl[sl], 16)
                        else:
                            for nn in range(nnsub):
                                sync.wait_ge(s_act, g + nn + 1)
                                sync.dma_start(
                                    out=out_d[msl,
                                              ng * ng_w + nn * NT:
                                              ng * ng_w + (nn + 1) * NT],
                                    in_=o_sb[:, sl * ng_w + nn * NT:
                                             sl * ng_w + (nn + 1) * NT]
                                ).then_inc(s_osl[sl], 16)
                # the end-of-block DRAIN quiesces the DGE queues, so no
                # explicit waits on the final transfer completions here

            @block.tensor
            def _(tensor):
                tensor.wait_ge(s_scr, 1)
                for _ in range(NWARM):
                    tensor.matmul(ps[:, 0:P], lhsT=scr_sb[:, P:2 * P],
                                  rhs=scr_sb[:, 0:P], start=True, stop=True)
                tensor.wait_ge(s_xT[0], 16)
                for ng in range(ngroups):
                    for m in range(mt):
                        g = ng * mt + m
                        sl = g % S_SLOTS
                        if ng == 0 and m == 1 and mt > 1:
                            tensor.wait_ge(s_xT[1], 16)
                        if ng == 0 and m == 2 and mt > 2:
                            tensor.wait_ge(s_xT[2], 16)
                        if g >= S_SLOTS:
                            tensor.wait_ge(s_dve, g - S_SLOTS + 1)
                        for nn in range(nnsub):
                            if m == 0:
                                b = ng * nnsub + nn
                                tensor.wait_ge(s_yb[b], yb_cnt[b])
                            for k in range(kt):
                                inst = tensor.matmul(
                                    ps[:, sl * ng_w + nn * NT:
                                       sl * ng_w + (nn + 1) * NT],
                                    lhsT=lhsT(k, m),
                                    rhs=rhs(k, ng, nn),
                                    start=(k == 0),
                                    stop=(k == kt - 1),
                                )
                        inst.then_inc(s_mm, 1)

            @block.vector
            def _(vector):
                vector.memset(scr_sb[:], 0.0).then_inc(s_scr, 1)
                for ng in range(ngroups):
                    gsl = slice(ng * ng_w, (ng + 1) * ng_w)
                    for m in range(mt):
                        g = ng * mt + m
                        sl = g % S_SLOTS
                        ssl = g % SS_SLOTS
                        vector.wait_ge(s_mm, g + 1)
                        if m == 0:
                            vector.wait_ge(s_y2c[ng], 16)
                        if g >= SS_SLOTS:
                            vector.wait_ge(s_act, g - SS_SLOTS + 1)
                        if g < G - 1:
                            vector.tensor_add(
                                s_sb[:, ssl * ng_w:(ssl + 1) * ng_w],
                                ps[:, sl * ng_w:(sl + 1) * ng_w],
                                y2_sb[:, gsl]).then_inc(s_dve, 1)
                        else:
                            # split the final drain chain to shorten the tail
                            for nn in range(nnsub):
                                vector.tensor_add(
                                    s_sb[:, ssl * ng_w + nn * NT:
                                         ssl * ng_w + (nn + 1) * NT],
                                    ps[:, sl * ng_w + nn * NT:
                                       sl * ng_w + (nn + 1) * NT],
                                    y2_sb[:, ng * ng_w + nn * NT:
                                          ng * ng_w + (nn + 1) * NT]
                                ).then_inc(s_dve, 1)

            @block.scalar
            def _(scalar):
                scalar.wait_ge(s_x2, 16)
                for ng in range(ngroups):
                    for m in range(mt):
                        g = ng * mt + m
                        ssl = g % SS_SLOTS
                        osl = g % O_SLOTS
                        q = (g - osl) // O_SLOTS
                        if q >= 1:
                            scalar.wait_ge(s_osl[osl], 16 * q)
                        if g < G - 1:
                            scalar.wait_ge(s_dve, g + 1)
                            scalar.activation(
                                o_sb[:, osl * ng_w:(osl + 1) * ng_w],
                                s_sb[:, ssl * ng_w:(ssl + 1) * ng_w],
                                mybir.ActivationFunctionType.Exp,
                                bias=x2_sb[:, m:m + 1],
                                scale=float(scale2g)).then_inc(s_act, 1)
                        else:
                            for nn in range(nnsub):
                                scalar.wait_ge(s_dve, g + nn + 1)
                                scalar.activation(
                                    o_sb[:, osl * ng_w + nn * NT:
                                         osl * ng_w + (nn + 1) * NT],
                                    s_sb[:, ssl * ng_w + nn * NT:
                                         ssl * ng_w + (nn + 1) * NT],
                                    mybir.ActivationFunctionType.Exp,
                                    bias=x2_sb[:, m:m + 1],
                                    scale=float(scale2g)).then_inc(s_act, 1)

        nc.compile()
    return nc


def _pack_xT(x_b: np.ndarray) -> np.ndarray:
    """[MC, D] bf16 -> SBUF image [128, MT*KT*128], block (m,k) at col
    (m*KT+k)*128 with element [p, c] = x[m*128 + c, k*128 + p]."""
    mcc, d = x_b.shape
    mt, kt = mcc // P, d // P
    a = x_b.reshape(mt, P, kt, P)          # [m, c, k, p]
    a = a.transpose(3, 0, 2, 1)            # [p, m, k, c]
    return np.ascontiguousarray(a.reshape(P, mt * kt * P))


def _pack_yT(y_b: np.ndarray, cw: int) -> np.ndarray:
    """[M, D] bf16 -> SBUF image [128, (M//cw)*KT*cw], block (b,k) at col
    (b*KT+k)*cw with element [p, c] = y[b*cw + c, k*128 + p]."""
    m, d = y_b.shape
    nb, kt = m // cw, d // P
    a = y_b.reshape(nb, cw, kt, P)         # [b, c, k, p]
    a = a.transpose(3, 0, 2, 1)            # [p, b, k, c]
    return np.ascontiguousarray(a.reshape(P, nb * kt * cw))


def kernel(x: np.ndarray, y: np.ndarray, gamma: np.ndarray) -> np.ndarray:
    from concourse.bass_utils import run_bass_kernel_spmd

    x = np.asarray(x, dtype=np.float32)
    y = np.asarray(y, dtype=np.float32)
    g = float(np.asarray(gamma))

    n, d = x.shape
    m = y.shape[0]
    assert (n, d, m) == (N_FULL, D, M_FULL), (n, d, m)

    raw = bool(int(os.environ.get("RBF_RAW", "1")))
    key = (g, n, d, m, raw)
    if key not in _cache:
        _cache.clear()
        build = _build_program_raw if raw else _build_program
        _cache[key] = build(2.0 * g, MC, M_FULL, D)
    nc = _cache[key]

    # host-side prep (O(N*D), ~0.01% of kernel FLOPs)
    bf16 = ml_dtypes.bfloat16
    x_b = x.astype(bf16)
    yTb = _pack_yT(y.astype(bf16), NT)
    y2 = np.einsum("md,md->m", y, y, dtype=np.float64)
    y2row = (-0.5 * y2).astype(np.float32)
    if raw:
        y2n = np.ascontiguousarray(np.broadcast_to(y2row, (P, m)))
    else:
        y2n = np.ascontiguousarray(y2row[None, :])
    x2 = np.einsum("nd,nd->n", x, x, dtype=np.float64)

    in_maps = []
    for c in range(N_CORES):
        sl = slice(c * MC, (c + 1) * MC)
        x2_c = np.ascontiguousarray(
            (-g * x2[sl]).astype(np.float32).reshape(MT, P).T)      # [128, MT]
        in_maps.append({"xTb": _pack_xT(x_b[sl]), "yTb": yTb,
                        "y2n": y2n, "x2b": x2_c})

    trace = bool(int(os.environ.get("RBF_TRACE", "0")))
    res = run_bass_kernel_spmd(nc, in_maps, core_ids=list(range(N_CORES)),
                               trace=trace)
    global LAST_RESULTS
    LAST_RESULTS = res
    return np.concatenate(
        [r["out"].astype(np.float32) for r in res.results], axis=0)


LAST_RESULTS = None



# revision 3
# speedup vs baseline: 1.4769x; 1.4769x over previous
# RBF Gram matrix kernel for Trainium2 (8 NeuronCores, SPMD).
#
# reference:  G[i, j] = exp(-gamma * ||x_i - y_j||^2)
#                    = exp(2*gamma*(x@y^T)[i,j] - gamma*||x_i||^2) * exp(-gamma*||y_j||^2)
#
# Sharding: row-shard x across 8 cores (1024 rows each), replicate y.
# Each core computes a [1024, 8192] slice of G:
#   PE   : xy = x_c @ y^T     fp8(e4m3) DoubleRow matmuls — K=512 as two
#          256-deep passes, 2 MACs/cell/cycle (~2x bf16 rate), fp32 PSUM
#   ACT  : o = Exp(2g*xy + (-g*||x||^2))  straight from PSUM (bias is the
#          per-partition x-norm vector, scale the 2*gamma immediate)
#   DVE  : o2 = o * exp(-g*||y||^2)   bf16*bf16 at 2x_1P rate
#   GPS  : one-time partition-broadcast of the exp(-g*||y||^2) row
#   DMA  : o2 tile (bf16) -> DRAM; host upcasts to fp32
#
# x^T / y^T are shipped pre-permuted fp8 images so each DoubleRow operand is
# a plain 3D AP [128, 2, cols] and every prefetch chunk is one contiguous DMA.
import os

import numpy as np
import ml_dtypes

N_CORES = 8
N_FULL = 8192          # rows of x (and of G)
M_FULL = 8192          # rows of y (cols of G)
D = 512                # feature dim (contraction)
MC = N_FULL // N_CORES # 1024 rows of x per core
P = 128                # SBUF partitions
NT = 512               # matmul moving tile (one fp32 psum bank)
KP = D // (2 * P)      # 2 DoubleRow k-passes (256 contraction each)
MT = MC // P           # 8 m-tiles per core
NB = M_FULL // NT      # 16 n-blocks of 512
NG = 2048              # psum slot width: 4 banks
NGR = M_FULL // NG     # 4 n-groups
NNS = NG // NT         # 4 n-blocks per group

_cache = {}


def _build_program(scale2g: float, neg_g: float):
    """Raw-Bass build: explicit per-engine programs + hand-rolled semaphores."""
    from contextlib import ExitStack, contextmanager

    import concourse.bass as bass
    import concourse.mybir as mybir
    from concourse import bacc

    class _NoBarrierBlock(bass.BassBlock):
        """BassBlock whose exit emits per-engine drains but no all-engine
        barrier; cross-engine ordering is fully covered by our semaphores."""

        def __exit__(self, exc_type, exc_val, exc_tb):
            if exc_type is not None:
                return
            for engine, last_body in self.last_body.items():
                with self.bass.body(last_body, parent=self.bass.cur_bb,
                                    allow_existing_parent=True):
                    engine.br(self.end_bb)
            self.bass.switch_bb(self.end_bb)
            gpsimd_type = self.bass.gpsimd.engine
            for eng_type, eng in self.bass.engines.items():
                if eng_type == gpsimd_type:
                    continue
                dr = mybir.InstDrain(
                    name=self.bass.get_next_instruction_name(),
                    ins=[], outs=[], bass_is_fusable=False)
                dr.engine = eng_type
                eng.add_instruction(dr)

    @contextmanager
    def _no_barrier_block(nc):
        assert nc.cur_block is None
        blk = _NoBarrierBlock(nc, f"block_{nc.next_id()}")
        nc.cur_block = blk
        try:
            with blk:
                yield blk
        finally:
            nc.cur_block = None

    DR = mybir.MatmulPerfMode.DoubleRow
    G = NGR * MT           # 32 pipeline groups of [128, 2048]
    O_SLOTS = 4
    NWARM = 40             # >=3.4us of cold N=128 matmuls warms the HAM gate

    nc = bacc.Bacc("TRN2", target_bir_lowering=False, debug=False,
                   num_devices=N_CORES)

    # exact SBUF images (see kernel() for the host-side permutes)
    xT_d = nc.dram_tensor("xTq", [P, 2 * MT, 2, P], mybir.dt.float8e4,
                          kind="ExternalInput").ap()
    yT_d = nc.dram_tensor("yTq", [P, 2 * NB, 2, NT], mybir.dt.float8e4,
                          kind="ExternalInput").ap()
    ey_d = nc.dram_tensor("eyb", [1, M_FULL], mybir.dt.bfloat16,
                          kind="ExternalInput").ap()
    x2_d = nc.dram_tensor("x2b", [P, MT], mybir.dt.float32,
                          kind="ExternalInput").ap()
    out_d = nc.dram_tensor("out", [MC, M_FULL], mybir.dt.bfloat16,
                           kind="ExternalOutput").ap()

    with ExitStack() as ctx:
        ec = ctx.enter_context
        xT_sb = ec(nc.sbuf_tensor([P, 2 * MT, 2, P], mybir.dt.float8e4))
        yT_sb = ec(nc.sbuf_tensor([P, 2 * NB, 2, NT], mybir.dt.float8e4))
        eyr_sb = ec(nc.sbuf_tensor([1, M_FULL], mybir.dt.bfloat16))
        ey_sb = ec(nc.sbuf_tensor([P, M_FULL], mybir.dt.bfloat16))
        x2_sb = ec(nc.sbuf_tensor([P, MT], mybir.dt.float32))
        scr_sb = ec(nc.sbuf_tensor([P, 2 * P], mybir.dt.bfloat16))
        o_sb = ec(nc.sbuf_tensor([P, O_SLOTS, NG], mybir.dt.bfloat16))
        o2_sb = ec(nc.sbuf_tensor([P, O_SLOTS, NG], mybir.dt.bfloat16))
        ps = ec(nc.psum_tensor([P, 2, NG], mybir.dt.float32))

        s_scr = ec(nc.semaphore(name="s_scr"))
        s_xT = ec(nc.semaphore(name="s_xT"))
        s_yb = [ec(nc.semaphore(name=f"s_yb{i}")) for i in range(NB // 2)]
        s_x2 = ec(nc.semaphore(name="s_x2"))
        s_ey = ec(nc.semaphore(name="s_ey"))
        s_eyb = ec(nc.semaphore(name="s_eyb"))
        s_mm = ec(nc.semaphore(name="s_mm"))
        s_act = ec(nc.semaphore(name="s_act"))
        s_dve = ec(nc.semaphore(name="s_dve"))
        s_osl = [ec(nc.semaphore(name=f"s_osl{i}")) for i in range(O_SLOTS)]

        with _no_barrier_block(nc) as block:

            @block.sync
            def _(sync):
                # startup set, in critical-path order. chunk c = 2 n-blocks.
                def dma_chunk(c):
                    sync.dma_start(out=yT_sb[:, 4 * c:4 * c + 4],
                                   in_=yT_d[:, 4 * c:4 * c + 4]
                                   ).then_inc(s_yb[c], 16)

                dma_chunk(0)
                sync.dma_start(out=xT_sb[:], in_=xT_d).then_inc(s_xT, 16)
                dma_chunk(1)
                sync.dma_start(out=x2_sb[:], in_=x2_d).then_inc(s_x2, 16)
                sync.dma_start(out=eyr_sb[:], in_=ey_d).then_inc(s_ey, 16)
                for c in range(2, NB // 2):
                    dma_chunk(c)
                for g in range(G):
                    ng, m = g // MT, g % MT
                    sl = g % O_SLOTS
                    msl = slice(m * P, (m + 1) * P)
                    if g < G - 1:
                        sync.wait_ge(s_dve, g + 1)
                        sync.dma_start(
                            out=out_d[msl, ng * NG:(ng + 1) * NG],
                            in_=o2_sb[:, sl]).then_inc(s_osl[sl], 16)
                    else:
                        # split the final drain chain to shorten the tail
                        for nn in range(NNS):
                            sync.wait_ge(s_dve, g + nn + 1)
                            nsl = slice(ng * NG + nn * NT,
                                        ng * NG + (nn + 1) * NT)
                            sync.dma_start(
                                out=out_d[msl, nsl],
                                in_=o2_sb[:, sl, nn * NT:(nn + 1) * NT]
                            ).then_inc(s_osl[sl], 16)
                # the end-of-block DRAIN quiesces the DGE queues, so no
                # explicit waits on the final transfer completions here

            @block.tensor
            def _(tensor):
                tensor.wait_ge(s_scr, 1)
                for _ in range(NWARM):
                    tensor.matmul(ps[:, 0, 0:P], lhsT=scr_sb[:, P:2 * P],
                                  rhs=scr_sb[:, 0:P], start=True, stop=True)
                tensor.wait_ge(s_xT, 16)
                for g in range(G):
                    ng, m = g // MT, g % MT
                    sl = g % 2
                    if m == 0:
                        tensor.wait_ge(s_yb[2 * ng], 16)
                        tensor.wait_ge(s_yb[2 * ng + 1], 16)
                    if g >= 2:
                        tensor.wait_ge(s_act, g - 1)
                    last = g == G - 1
                    for kp in range(KP):
                        for nn in range(NNS):
                            inst = tensor.matmul(
                                ps[:, sl, nn * NT:(nn + 1) * NT],
                                lhsT=xT_sb[:, 2 * m + kp],
                                rhs=yT_sb[:, (NNS * ng + nn) * 2 + kp],
                                start=(kp == 0),
                                stop=(kp == KP - 1),
                                perf_mode=DR,
                            )
                            if last and kp == KP - 1:
                                inst.then_inc(s_mm, 1)
                    if not last:
                        inst.then_inc(s_mm, 1)

            @block.scalar
            def _(scalar):
                scalar.wait_ge(s_x2, 16)
                for g in range(G):
                    ng, m = g // MT, g % MT
                    sl = g % 2
                    osl = g % O_SLOTS
                    if g >= O_SLOTS:
                        scalar.wait_ge(s_dve, g - O_SLOTS + 1)
                    if g < G - 1:
                        scalar.wait_ge(s_mm, g + 1)
                        scalar.activation(
                            o_sb[:, osl], ps[:, sl],
                            mybir.ActivationFunctionType.Exp,
                            bias=x2_sb[:, m:m + 1],
                            scale=float(scale2g)).then_inc(s_act, 1)
                    else:
                        # split the final drain chain to shorten the tail
                        for nn in range(NNS):
                            scalar.wait_ge(s_mm, g + nn + 1)
                            scalar.activation(
                                o_sb[:, osl, nn * NT:(nn + 1) * NT],
                                ps[:, sl, nn * NT:(nn + 1) * NT],
                                mybir.ActivationFunctionType.Exp,
                                bias=x2_sb[:, m:m + 1],
                                scale=float(scale2g)).then_inc(s_act, 1)

            @block.vector
            def _(vector):
                vector.memset(scr_sb[:], 0.0).then_inc(s_scr, 1)
                for g in range(G):
                    ng, m = g // MT, g % MT
                    osl = g % O_SLOTS
                    gsl = slice(ng * NG, (ng + 1) * NG)
                    if g == 0 or (g % MT == 0):
                        vector.wait_ge(s_eyb, ng + 1)
                    if g >= O_SLOTS:
                        vector.wait_ge(s_osl[osl], 16 * (g // O_SLOTS))
                    if g < G - 1:
                        vector.wait_ge(s_act, g + 1)
                        vector.tensor_mul(o2_sb[:, osl], o_sb[:, osl],
                                          ey_sb[:, gsl]).then_inc(s_dve, 1)
                    else:
                        for nn in range(NNS):
                            vector.wait_ge(s_act, g + nn + 1)
                            nsl = slice(ng * NG + nn * NT,
                                        ng * NG + (nn + 1) * NT)
                            vector.tensor_mul(
                                o2_sb[:, osl, nn * NT:(nn + 1) * NT],
                                o_sb[:, osl, nn * NT:(nn + 1) * NT],
                                ey_sb[:, nsl]).then_inc(s_dve, 1)

            @block.gpsimd
            def _(gpsimd):
                gpsimd.wait_ge(s_ey, 16)
                for c in range(NGR):
                    gsl = slice(c * NG, (c + 1) * NG)
                    gpsimd.partition_broadcast(
                        ey_sb[:, gsl], eyr_sb[0:1, gsl]).then_inc(s_eyb, 1)

        nc.compile()
    return nc


def _pack_xT(xq: np.ndarray) -> np.ndarray:
    """[MC, D] fp8 -> [128, 2*MT, 2, 128]; [p, 2m+kp, s, c] =
    x[m*128 + c, kp*256 + s*128 + p]."""
    a = xq.reshape(MT, P, KP, 2, P)        # [m, c, kp, s, p]
    a = a.transpose(4, 0, 2, 3, 1)         # [p, m, kp, s, c]
    return np.ascontiguousarray(a.reshape(P, 2 * MT, 2, P))


def _pack_yT(yq: np.ndarray) -> np.ndarray:
    """[M, D] fp8 -> [128, 2*NB, 2, NT]; [p, 2nb+kp, s, c] =
    y[nb*512 + c, kp*256 + s*128 + p]."""
    a = yq.reshape(NB, NT, KP, 2, P)       # [nb, c, kp, s, p]
    a = a.transpose(4, 0, 2, 3, 1)         # [p, nb, kp, s, c]
    return np.ascontiguousarray(a.reshape(P, 2 * NB, 2, NT))


def kernel(x: np.ndarray, y: np.ndarray, gamma: np.ndarray) -> np.ndarray:
    from concourse.bass_utils import run_bass_kernel_spmd

    x = np.asarray(x, dtype=np.float32)
    y = np.asarray(y, dtype=np.float32)
    g = float(np.asarray(gamma))

    n, d = x.shape
    m = y.shape[0]
    assert (n, d, m) == (N_FULL, D, M_FULL), (n, d, m)

    key = g
    if key not in _cache:
        _cache.clear()
        _cache[key] = _build_program(2.0 * g, -g)
    nc = _cache[key]

    # host-side prep (O(N*D), ~0.01% of kernel FLOPs)
    fp8 = ml_dtypes.float8_e4m3
    bf16 = ml_dtypes.bfloat16
    yTq = _pack_yT(y.astype(fp8))
    y2 = np.einsum("md,md->m", y, y, dtype=np.float64)
    eyb = np.exp(-g * y2).astype(bf16)[None, :]
    x2 = np.einsum("nd,nd->n", x, x, dtype=np.float64)

    in_maps = []
    for c in range(N_CORES):
        sl = slice(c * MC, (c + 1) * MC)
        x2_c = np.ascontiguousarray(
            (-g * x2[sl]).astype(np.float32).reshape(MT, P).T)      # [128, MT]
        in_maps.append({"xTq": _pack_xT(x[sl].astype(fp8)), "yTq": yTq,
                        "eyb": eyb, "x2b": x2_c})

    trace = bool(int(os.environ.get("RBF_TRACE", "0")))
    res = run_bass_kernel_spmd(nc, in_maps, core_ids=list(range(N_CORES)),
                               trace=trace)
    global LAST_RESULTS
    LAST_RESULTS = res
    return np.concatenate(
        [r["out"].astype(np.float32) for r in res.results], axis=0)


LAST_RESULTS = None


# revision 6
# speedup vs baseline: 1.5349x; 1.0393x over previous
# RBF Gram matrix kernel for Trainium2 (8 NeuronCores, SPMD).
#
# reference:  G[i, j] = exp(-gamma * ||x_i - y_j||^2)
#                    = exp(2*gamma*(x@y^T)[i,j] - gamma*||x_i||^2) * exp(-gamma*||y_j||^2)
#
# Sharding: row-shard x across 8 cores (1024 rows each), replicate y.
# Each core computes a [1024, 8192] slice of G:
#   PE   : xy = x_c @ y^T     fp8(e4m3) DoubleRow matmuls — K=512 as two
#          256-deep passes, 2 MACs/cell/cycle (~2x bf16 rate), fp32 PSUM
#   ACT  : o = Exp(2g*xy + (-g*||x||^2))  straight from PSUM (bias is the
#          per-partition x-norm vector, scale the 2*gamma immediate)
#   DVE  : o2 = o * exp(-g*||y||^2)   bf16*bf16 at 2x_1P rate
#   DMA  : o2 tile (bf16) -> DRAM; host upcasts to fp32
#
# The steady state is ACT-bound (~2.0us per [128,2048] group vs 1.73us PE),
# so the schedule aims ACT back-to-back from ~11us: fine-grained startup DMAs
# (first n-blocks as 256KB singles), a ~2.6us PE warmup that hands off to
# DMA-paced real matmuls with no >3.4us gap (keeps the HAM clock-gate warm),
# group 0 split at 512-wide grain to fill the ACT pipe early, and a split
# tail whose last DMAs are issued from both SP and ACT queues.
import os

import numpy as np
import ml_dtypes

N_CORES = 8
N_FULL = 8192          # rows of x (and of G)
M_FULL = 8192          # rows of y (cols of G)
D = 512                # feature dim (contraction)
MC = N_FULL // N_CORES # 1024 rows of x per core
P = 128                # SBUF partitions
NT = 512               # matmul moving tile (one fp32 psum bank)
KP = D // (2 * P)      # 2 DoubleRow k-passes (256 contraction each)
MT = MC // P           # 8 m-tiles per core
NB = M_FULL // NT      # 16 n-blocks of 512
NG = 2048              # psum slot width: 4 banks
NGR = M_FULL // NG     # 4 n-groups
NNS = NG // NT         # 4 n-blocks per group

_cache = {}


def _build_program(scale2g: float):
    """Raw-Bass build: explicit per-engine programs + hand-rolled semaphores."""
    from contextlib import ExitStack, contextmanager

    import concourse.bass as bass
    import concourse.mybir as mybir
    from concourse import bacc

    class _NoBarrierBlock(bass.BassBlock):
        """BassBlock whose exit emits per-engine drains but no all-engine
        barrier; cross-engine ordering is fully covered by our semaphores."""

        def __exit__(self, exc_type, exc_val, exc_tb):
            if exc_type is not None:
                return
            for engine, last_body in self.last_body.items():
                with self.bass.body(last_body, parent=self.bass.cur_bb,
                                    allow_existing_parent=True):
                    engine.br(self.end_bb)
            self.bass.switch_bb(self.end_bb)
            gpsimd_type = self.bass.gpsimd.engine
            for eng_type, eng in self.bass.engines.items():
                if eng_type == gpsimd_type:
                    continue
                dr = mybir.InstDrain(
                    name=self.bass.get_next_instruction_name(),
                    ins=[], outs=[], bass_is_fusable=False)
                dr.engine = eng_type
                eng.add_instruction(dr)

    @contextmanager
    def _no_barrier_block(nc):
        assert nc.cur_block is None
        blk = _NoBarrierBlock(nc, f"block_{nc.next_id()}")
        nc.cur_block = blk
        try:
            with blk:
                yield blk
        finally:
            nc.cur_block = None

    DR = mybir.MatmulPerfMode.DoubleRow
    G = NGR * MT           # 32 pipeline groups of [128, 2048]
    O_SLOTS = 4
    NWARM = 24             # ~2.6us of cold N=128 matmuls primes the HAM gate

    nc = bacc.Bacc("TRN2", target_bir_lowering=False, debug=False,
                   num_devices=N_CORES)

    # exact SBUF images (see kernel() for the host-side permutes)
    xT_d = nc.dram_tensor("xTq", [P, 2 * MT, 2, P], mybir.dt.float8e4,
                          kind="ExternalInput").ap()
    yT_d = nc.dram_tensor("yTq", [P, 2 * NB, 2, NT], mybir.dt.float8e4,
                          kind="ExternalInput").ap()
    ey_d = nc.dram_tensor("eyb", [P, M_FULL], mybir.dt.bfloat16,
                          kind="ExternalInput").ap()
    x2_d = nc.dram_tensor("x2b", [P, MT], mybir.dt.float32,
                          kind="ExternalInput").ap()
    out_d = nc.dram_tensor("out", [MC, M_FULL], mybir.dt.bfloat16,
                           kind="ExternalOutput").ap()

    with ExitStack() as ctx:
        ec = ctx.enter_context
        xT_sb = ec(nc.sbuf_tensor([P, 2 * MT, 2, P], mybir.dt.float8e4))
        yT_sb = ec(nc.sbuf_tensor([P, 2 * NB, 2, NT], mybir.dt.float8e4))
        ey_sb = ec(nc.sbuf_tensor([P, M_FULL], mybir.dt.bfloat16))
        x2_sb = ec(nc.sbuf_tensor([P, MT], mybir.dt.float32))
        scr_sb = ec(nc.sbuf_tensor([P, 2 * P], mybir.dt.bfloat16))
        o_sb = ec(nc.sbuf_tensor([P, O_SLOTS, NG], mybir.dt.bfloat16))
        o2_sb = ec(nc.sbuf_tensor([P, O_SLOTS, NG], mybir.dt.bfloat16))
        ps = ec(nc.psum_tensor([P, 2, NG], mybir.dt.float32))

        s_scr = ec(nc.semaphore(name="s_scr"))
        s_xm0 = ec(nc.semaphore(name="s_xm0"))
        s_xr = ec(nc.semaphore(name="s_xr"))
        s_yb = [ec(nc.semaphore(name=f"s_yb{i}")) for i in range(4)]
        s_ybp = [ec(nc.semaphore(name=f"s_ybp{i}")) for i in range(6)]
        s_x2 = ec(nc.semaphore(name="s_x2"))
        s_ey = [ec(nc.semaphore(name=f"s_ey{i}")) for i in range(2)]
        s_mm = ec(nc.semaphore(name="s_mm"))
        s_act = ec(nc.semaphore(name="s_act"))
        s_dve = ec(nc.semaphore(name="s_dve"))
        s_osl = [ec(nc.semaphore(name=f"s_osl{i}")) for i in range(O_SLOTS)]

        # completion counts: group 0 and group G-1 are split into 4 chunks,
        # groups 1..G-2 are single units.  after group g completes:
        #   s_mm/s_act/s_dve == g + 4
        def wait_yb(eng, blk):
            # n-block blk of y^T resident? first 4 blocks ship as singles,
            # the rest as pairs
            if blk < 4:
                eng.wait_ge(s_yb[blk], 16)
            else:
                eng.wait_ge(s_ybp[blk // 2 - 2], 16)

        with _no_barrier_block(nc) as block:

            @block.sync
            def _(sync):
                # startup set, in critical-path order
                sync.dma_start(out=xT_sb[:, 0:2], in_=xT_d[:, 0:2]
                               ).then_inc(s_xm0, 16)
                for b in range(4):
                    sync.dma_start(out=yT_sb[:, 2 * b:2 * b + 2],
                                   in_=yT_d[:, 2 * b:2 * b + 2]
                                   ).then_inc(s_yb[b], 16)
                sync.dma_start(out=x2_sb[:], in_=x2_d).then_inc(s_x2, 16)
                sync.dma_start(out=xT_sb[:, 2:], in_=xT_d[:, 2:]
                               ).then_inc(s_xr, 16)
                for p in range(2):  # blocks 4..7
                    sync.dma_start(out=yT_sb[:, 8 + 4 * p:12 + 4 * p],
                                   in_=yT_d[:, 8 + 4 * p:12 + 4 * p]
                                   ).then_inc(s_ybp[p], 16)
                sync.dma_start(out=ey_sb[:, 0:M_FULL // 2],
                               in_=ey_d[:, 0:M_FULL // 2]).then_inc(s_ey[0], 16)
                for p in range(2, 6):  # blocks 8..15
                    sync.dma_start(out=yT_sb[:, 8 + 4 * p:12 + 4 * p],
                                   in_=yT_d[:, 8 + 4 * p:12 + 4 * p]
                                   ).then_inc(s_ybp[p], 16)
                sync.dma_start(out=ey_sb[:, M_FULL // 2:],
                               in_=ey_d[:, M_FULL // 2:]).then_inc(s_ey[1], 16)
                for g in range(G):
                    ng, m = g // MT, g % MT
                    sl = g % O_SLOTS
                    msl = slice(m * P, (m + 1) * P)
                    if g < G - 1:
                        sync.wait_ge(s_dve, g + 4)
                        sync.dma_start(
                            out=out_d[msl, ng * NG:(ng + 1) * NG],
                            in_=o2_sb[:, sl]).then_inc(s_osl[sl], 16)
                    else:
                        # tail: nn0/nn1 from here, nn2/nn3 from the ACT queue
                        for nn in range(2):
                            sync.wait_ge(s_dve, g + 4 + nn)
                            nsl = slice(ng * NG + nn * NT,
                                        ng * NG + (nn + 1) * NT)
                            sync.dma_start(
                                out=out_d[msl, nsl],
                                in_=o2_sb[:, sl, nn * NT:(nn + 1) * NT]
                            ).then_inc(s_osl[sl], 16)
                # the end-of-block DRAIN quiesces the DGE queues, so no
                # explicit waits on the final transfer completions here

            @block.tensor
            def _(tensor):
                tensor.wait_ge(s_scr, 1)
                for _ in range(NWARM):
                    tensor.matmul(ps[:, 0, 0:P], lhsT=scr_sb[:, P:2 * P],
                                  rhs=scr_sb[:, 0:P], start=True, stop=True)
                tensor.wait_ge(s_xm0, 16)
                # group 0: kp-inner per 512-block so ACT can start early
                for nn in range(NNS):
                    tensor.wait_ge(s_yb[nn], 16)
                    for kp in range(KP):
                        inst = tensor.matmul(
                            ps[:, 0, nn * NT:(nn + 1) * NT],
                            lhsT=xT_sb[:, kp],
                            rhs=yT_sb[:, 2 * nn + kp],
                            start=(kp == 0), stop=(kp == KP - 1),
                            perf_mode=DR)
                    inst.then_inc(s_mm, 1)
                for g in range(1, G):
                    ng, m = g // MT, g % MT
                    sl = g % 2
                    if g == 1:
                        tensor.wait_ge(s_xr, 16)
                    if g >= 2:
                        tensor.wait_ge(s_act, g + 2)
                    last = g == G - 1
                    for kp in range(KP):
                        for nn in range(NNS):
                            if kp == 0 and m == 0:
                                wait_yb(tensor, NNS * ng + nn)
                            inst = tensor.matmul(
                                ps[:, sl, nn * NT:(nn + 1) * NT],
                                lhsT=xT_sb[:, 2 * m + kp],
                                rhs=yT_sb[:, (NNS * ng + nn) * 2 + kp],
                                start=(kp == 0),
                                stop=(kp == KP - 1),
                                perf_mode=DR,
                            )
                            if last and kp == KP - 1:
                                inst.then_inc(s_mm, 1)
                    if not last:
                        inst.then_inc(s_mm, 1)

            @block.scalar
            def _(scalar):
                EXP = mybir.ActivationFunctionType.Exp
                scalar.wait_ge(s_x2, 16)
                # group 0 at 512 grain
                for nn in range(NNS):
                    scalar.wait_ge(s_mm, nn + 1)
                    scalar.activation(
                        o_sb[:, 0, nn * NT:(nn + 1) * NT],
                        ps[:, 0, nn * NT:(nn + 1) * NT], EXP,
                        bias=x2_sb[:, 0:1],
                        scale=float(scale2g)).then_inc(s_act, 1)
                for g in range(1, G):
                    ng, m = g // MT, g % MT
                    sl = g % 2
                    osl = g % O_SLOTS
                    if g >= O_SLOTS:
                        scalar.wait_ge(s_dve, g)
                    if g < G - 1:
                        scalar.wait_ge(s_mm, g + 4)
                        scalar.activation(
                            o_sb[:, osl], ps[:, sl], EXP,
                            bias=x2_sb[:, m:m + 1],
                            scale=float(scale2g)).then_inc(s_act, 1)
                    else:
                        for nn in range(NNS):
                            scalar.wait_ge(s_mm, g + 4 + nn)
                            scalar.activation(
                                o_sb[:, osl, nn * NT:(nn + 1) * NT],
                                ps[:, sl, nn * NT:(nn + 1) * NT], EXP,
                                bias=x2_sb[:, m:m + 1],
                                scale=float(scale2g)).then_inc(s_act, 1)
                        # tail DMAs for nn2/nn3 (SP covers nn0/nn1)
                        msl = slice(m * P, (m + 1) * P)
                        for nn in (2, 3):
                            scalar.wait_ge(s_dve, g + 4 + nn)
                            nsl = slice(ng * NG + nn * NT,
                                        ng * NG + (nn + 1) * NT)
                            scalar.dma_start(
                                out=out_d[msl, nsl],
                                in_=o2_sb[:, osl, nn * NT:(nn + 1) * NT]
                            ).then_inc(s_osl[osl], 16)

            @block.vector
            def _(vector):
                vector.memset(scr_sb[:], 0.0).then_inc(s_scr, 1)
                vector.wait_ge(s_ey[0], 16)
                for nn in range(NNS):
                    vector.wait_ge(s_act, nn + 1)
                    vector.tensor_mul(
                        o2_sb[:, 0, nn * NT:(nn + 1) * NT],
                        o_sb[:, 0, nn * NT:(nn + 1) * NT],
                        ey_sb[:, nn * NT:(nn + 1) * NT]).then_inc(s_dve, 1)
                for g in range(1, G):
                    ng, m = g // MT, g % MT
                    osl = g % O_SLOTS
                    gsl = slice(ng * NG, (ng + 1) * NG)
                    if g == 2 * MT:  # first group of the upper ey half
                        vector.wait_ge(s_ey[1], 16)
                    if g >= O_SLOTS:
                        vector.wait_ge(s_osl[osl], 16 * (g // O_SLOTS))
                    if g < G - 1:
                        vector.wait_ge(s_act, g + 4)
                        vector.tensor_mul(o2_sb[:, osl], o_sb[:, osl],
                                          ey_sb[:, gsl]).then_inc(s_dve, 1)
                    else:
                        for nn in range(NNS):
                            vector.wait_ge(s_act, g + 4 + nn)
                            nsl = slice(ng * NG + nn * NT,
                                        ng * NG + (nn + 1) * NT)
                            vector.tensor_mul(
                                o2_sb[:, osl, nn * NT:(nn + 1) * NT],
                                o_sb[:, osl, nn * NT:(nn + 1) * NT],
                                ey_sb[:, nsl]).then_inc(s_dve, 1)

        nc.compile()
    return nc


def _pack_xT(xq: np.ndarray) -> np.ndarray:
    """[MC, D] fp8 -> [128, 2*MT, 2, 128]; [p, 2m+kp, s, c] =
    x[m*128 + c, kp*256 + s*128 + p]."""
    a = xq.reshape(MT, P, KP, 2, P)        # [m, c, kp, s, p]
    a = a.transpose(4, 0, 2, 3, 1)         # [p, m, kp, s, c]
    return np.ascontiguousarray(a.reshape(P, 2 * MT, 2, P))


def _pack_yT(yq: np.ndarray) -> np.ndarray:
    """[M, D] fp8 -> [128, 2*NB, 2, NT]; [p, 2nb+kp, s, c] =
    y[nb*512 + c, kp*256 + s*128 + p]."""
    a = yq.reshape(NB, NT, KP, 2, P)       # [nb, c, kp, s, p]
    a = a.transpose(4, 0, 2, 3, 1)         # [p, nb, kp, s, c]
    return np.ascontiguousarray(a.reshape(P, 2 * NB, 2, NT))


def kernel(x: np.ndarray, y: np.ndarray, gamma: np.ndarray) -> np.ndarray:
    from concourse.bass_utils import run_bass_kernel_spmd

    x = np.asarray(x, dtype=np.float32)
    y = np.asarray(y, dtype=np.float32)
    g = float(np.asarray(gamma))

    n, d = x.shape
    m = y.shape[0]
    assert (n, d, m) == (N_FULL, D, M_FULL), (n, d, m)

    key = g
    if key not in _cache:
        _cache.clear()
        _cache[key] = _build_program(2.0 * g)
    nc = _cache[key]

    # host-side prep (O(N*D), ~0.01% of kernel FLOPs)
    fp8 = ml_dtypes.float8_e4m3
    bf16 = ml_dtypes.bfloat16
    yTq = _pack_yT(y.astype(fp8))
    y2 = np.einsum("md,md->m", y, y, dtype=np.float64)
    ey_row = np.exp(-g * y2).astype(bf16)
    eyb = np.ascontiguousarray(np.broadcast_to(ey_row, (P, M_FULL)))
    x2 = np.einsum("nd,nd->n", x, x, dtype=np.float64)

    in_maps = []
    for c in range(N_CORES):
        sl = slice(c * MC, (c + 1) * MC)
        x2_c = np.ascontiguousarray(
            (-g * x2[sl]).astype(np.float32).reshape(MT, P).T)      # [128, MT]
        in_maps.append({"xTq": _pack_xT(x[sl].astype(fp8)), "yTq": yTq,
                        "eyb": eyb, "x2b": x2_c})

    trace = bool(int(os.environ.get("RBF_TRACE", "0")))
    res = run_bass_kernel_spmd(nc, in_maps, core_ids=list(range(N_CORES)),
                               trace=trace)
    global LAST_RESULTS
    LAST_RESULTS = res
    return np.concatenate(
        [r["out"].astype(np.float32) for r in res.results], axis=0)


LAST_RESULTS = None


# revision 10
# speedup vs baseline: 1.5602x; 1.0165x over previous
# RBF Gram matrix kernel for Trainium2 (8 NeuronCores, SPMD).
#
# reference:  G[i, j] = exp(-gamma * ||x_i - y_j||^2)
#                    = exp(2*gamma*(x@y^T)[i,j] - gamma*||x_i||^2) * exp(-gamma*||y_j||^2)
#
# Sharding: row-shard x across 8 cores (1024 rows each), replicate y.
# Each core computes a [1024, 8192] slice of G:
#   PE   : xy = x_c @ y^T     fp8(e4m3) DoubleRow matmuls — K=512 as two
#          256-deep passes, 2 MACs/cell/cycle (~2x bf16 rate), fp32 PSUM
#   ACT  : o = Exp(2g*xy + (-g*||x||^2))  straight from PSUM (bias is the
#          per-partition x-norm vector, scale the 2*gamma immediate)
#   DVE  : o2 = o * exp(-g*||y||^2)   bf16*bf16 at 2x_1P rate
#   DMA  : o2 tile (bf16) -> DRAM; host upcasts to fp32
#
# The steady state is ACT-bound (~2.0us per [128,2048] group vs 1.73us PE),
# so the schedule aims ACT back-to-back from ~11us: fine-grained startup DMAs
# (first n-blocks as 256KB singles), a ~2.6us PE warmup that hands off to
# DMA-paced real matmuls with no >3.4us gap (keeps the HAM clock-gate warm),
# group 0 split at 512-wide grain to fill the ACT pipe early, and a split
# tail whose last DMAs are issued from both SP and ACT queues.
import os

import numpy as np
import ml_dtypes

N_CORES = 8
N_FULL = 8192          # rows of x (and of G)
M_FULL = 8192          # rows of y (cols of G)
D = 512                # feature dim (contraction)
MC = N_FULL // N_CORES # 1024 rows of x per core
P = 128                # SBUF partitions
NT = 512               # matmul moving tile (one fp32 psum bank)
KP = D // (2 * P)      # 2 DoubleRow k-passes (256 contraction each)
MT = MC // P           # 8 m-tiles per core
NB = M_FULL // NT      # 16 n-blocks of 512
NG = 2048              # psum slot width: 4 banks
NGR = M_FULL // NG     # 4 n-groups
NNS = NG // NT         # 4 n-blocks per group

_cache = {}


def _build_program(scale2g: float):
    """Raw-Bass build: explicit per-engine programs + hand-rolled semaphores."""
    from contextlib import ExitStack, contextmanager

    import concourse.bass as bass
    import concourse.mybir as mybir
    from concourse import bacc

    class _NoBarrierBlock(bass.BassBlock):
        """BassBlock whose exit emits per-engine drains but no all-engine
        barrier; cross-engine ordering is fully covered by our semaphores."""

        def __exit__(self, exc_type, exc_val, exc_tb):
            if exc_type is not None:
                return
            for engine, last_body in self.last_body.items():
                with self.bass.body(last_body, parent=self.bass.cur_bb,
                                    allow_existing_parent=True):
                    engine.br(self.end_bb)
            self.bass.switch_bb(self.end_bb)
            gpsimd_type = self.bass.gpsimd.engine
            for eng_type, eng in self.bass.engines.items():
                if eng_type == gpsimd_type:
                    continue
                dr = mybir.InstDrain(
                    name=self.bass.get_next_instruction_name(),
                    ins=[], outs=[], bass_is_fusable=False)
                dr.engine = eng_type
                eng.add_instruction(dr)

    @contextmanager
    def _no_barrier_block(nc):
        assert nc.cur_block is None
        blk = _NoBarrierBlock(nc, f"block_{nc.next_id()}")
        nc.cur_block = blk
        try:
            with blk:
                yield blk
        finally:
            nc.cur_block = None

    DR = mybir.MatmulPerfMode.DoubleRow
    G = NGR * MT           # 32 pipeline groups of [128, 2048]
    O_SLOTS = 4
    NWARM = 24             # ~2.6us of cold N=128 matmuls primes the HAM gate

    nc = bacc.Bacc("TRN2", target_bir_lowering=False, debug=False,
                   num_devices=N_CORES)

    # exact SBUF images (see kernel() for the host-side permutes)
    xT_d = nc.dram_tensor("xTq", [P, 2 * MT, 2, P], mybir.dt.float8e4,
                          kind="ExternalInput").ap()
    yT_d = nc.dram_tensor("yTq", [P, 2 * NB, 2, NT], mybir.dt.float8e4,
                          kind="ExternalInput").ap()
    ey_d = nc.dram_tensor("eyb", [P, M_FULL], mybir.dt.bfloat16,
                          kind="ExternalInput").ap()
    x2_d = nc.dram_tensor("x2b", [P, MT], mybir.dt.float32,
                          kind="ExternalInput").ap()
    out_d = nc.dram_tensor("out", [MC, M_FULL], mybir.dt.bfloat16,
                           kind="ExternalOutput").ap()

    with ExitStack() as ctx:
        ec = ctx.enter_context
        xT_sb = ec(nc.sbuf_tensor([P, 2 * MT, 2, P], mybir.dt.float8e4))
        yT_sb = ec(nc.sbuf_tensor([P, 2 * NB, 2, NT], mybir.dt.float8e4))
        ey_sb = ec(nc.sbuf_tensor([P, M_FULL], mybir.dt.bfloat16))
        x2_sb = ec(nc.sbuf_tensor([P, MT], mybir.dt.float32))
        scr_sb = ec(nc.sbuf_tensor([P, 2 * P], mybir.dt.bfloat16))
        o_sb = ec(nc.sbuf_tensor([P, O_SLOTS, NG], mybir.dt.bfloat16))
        o2_sb = ec(nc.sbuf_tensor([P, O_SLOTS, NG], mybir.dt.bfloat16))
        ps = ec(nc.psum_tensor([P, 2, NG], mybir.dt.float32))

        s_scr = ec(nc.semaphore(name="s_scr"))
        s_xm0 = ec(nc.semaphore(name="s_xm0"))
        s_xr = ec(nc.semaphore(name="s_xr"))
        s_ybA = [ec(nc.semaphore(name=f"s_ybA{i}")) for i in range(2)]
        s_ybB = [ec(nc.semaphore(name=f"s_ybB{i}")) for i in range(3)]
        s_x2 = ec(nc.semaphore(name="s_x2"))
        s_ey = [ec(nc.semaphore(name=f"s_ey{i}")) for i in range(NGR)]
        s_mm = ec(nc.semaphore(name="s_mm"))
        s_act = ec(nc.semaphore(name="s_act"))
        s_dve = ec(nc.semaphore(name="s_dve"))
        s_osl = [ec(nc.semaphore(name=f"s_osl{i}")) for i in range(O_SLOTS)]

        # completion counts: group 0 and group G-1 are split into 4 chunks,
        # groups 1..G-2 are single units.  after group g completes:
        #   s_mm/s_act/s_dve == g + 4
        with _no_barrier_block(nc) as block:

            @block.sync
            def _(sync):
                # startup set, in critical-path order (the two 512KB y^T
                # chunks of n-group 0 and the first exp(-g*y^2) quarter ride
                # the ACT queue in parallel — see block.scalar)
                sync.dma_start(out=xT_sb[:, 0:2], in_=xT_d[:, 0:2]
                               ).then_inc(s_xm0, 16)
                sync.dma_start(out=x2_sb[:], in_=x2_d).then_inc(s_x2, 16)
                sync.dma_start(out=xT_sb[:, 2:], in_=xT_d[:, 2:]
                               ).then_inc(s_xr, 16)
                sync.dma_start(out=yT_sb[:, 8:16], in_=yT_d[:, 8:16]
                               ).then_inc(s_ybB[0], 16)
                sync.dma_start(out=ey_sb[:, NG:2 * NG],
                               in_=ey_d[:, NG:2 * NG]).then_inc(s_ey[1], 16)
                sync.dma_start(out=yT_sb[:, 16:24], in_=yT_d[:, 16:24]
                               ).then_inc(s_ybB[1], 16)
                sync.dma_start(out=ey_sb[:, 2 * NG:3 * NG],
                               in_=ey_d[:, 2 * NG:3 * NG]).then_inc(s_ey[2], 16)
                sync.dma_start(out=yT_sb[:, 24:32], in_=yT_d[:, 24:32]
                               ).then_inc(s_ybB[2], 16)
                sync.dma_start(out=ey_sb[:, 3 * NG:],
                               in_=ey_d[:, 3 * NG:]).then_inc(s_ey[3], 16)
                for g in range(G):
                    ng, m = g // MT, g % MT
                    sl = g % O_SLOTS
                    msl = slice(m * P, (m + 1) * P)
                    if g < G - 1:
                        sync.wait_ge(s_dve, g + 4)
                        sync.dma_start(
                            out=out_d[msl, ng * NG:(ng + 1) * NG],
                            in_=o2_sb[:, sl]).then_inc(s_osl[sl], 16)
                    else:
                        # tail: nn0/nn1 from here, nn2/nn3 from the ACT queue
                        for nn in range(2):
                            sync.wait_ge(s_dve, g + 4 + nn)
                            nsl = slice(ng * NG + nn * NT,
                                        ng * NG + (nn + 1) * NT)
                            sync.dma_start(
                                out=out_d[msl, nsl],
                                in_=o2_sb[:, sl, nn * NT:(nn + 1) * NT]
                            ).then_inc(s_osl[sl], 16)
                # the end-of-block DRAIN quiesces the DGE queues, so no
                # explicit waits on the final transfer completions here

            @block.tensor
            def _(tensor):
                tensor.wait_ge(s_scr, 1)
                for _ in range(NWARM):
                    tensor.matmul(ps[:, 0, 0:P], lhsT=scr_sb[:, P:2 * P],
                                  rhs=scr_sb[:, 0:P], start=True, stop=True)
                tensor.wait_ge(s_xm0, 16)
                # group 0: kp-inner per 512-block so ACT can start early
                for nn in range(NNS):
                    if nn in (0, 2):
                        tensor.wait_ge(s_ybA[nn // 2], 16)
                    for kp in range(KP):
                        inst = tensor.matmul(
                            ps[:, 0, nn * NT:(nn + 1) * NT],
                            lhsT=xT_sb[:, kp],
                            rhs=yT_sb[:, 2 * nn + kp],
                            start=(kp == 0), stop=(kp == KP - 1),
                            perf_mode=DR)
                    inst.then_inc(s_mm, 1)
                for g in range(1, G):
                    ng, m = g // MT, g % MT
                    sl = g % 2
                    if g == 1:
                        tensor.wait_ge(s_xr, 16)
                    if g >= 2:
                        tensor.wait_ge(s_act, g + 2)
                    if m == 0:
                        tensor.wait_ge(s_ybB[ng - 1], 16)
                    last = g == G - 1
                    for kp in range(KP):
                        for nn in range(NNS):
                            inst = tensor.matmul(
                                ps[:, sl, nn * NT:(nn + 1) * NT],
                                lhsT=xT_sb[:, 2 * m + kp],
                                rhs=yT_sb[:, (NNS * ng + nn) * 2 + kp],
                                start=(kp == 0),
                                stop=(kp == KP - 1),
                                perf_mode=DR,
                            )
                            if last and kp == KP - 1:
                                inst.then_inc(s_mm, 1)
                    if not last:
                        inst.then_inc(s_mm, 1)

            @block.scalar
            def _(scalar):
                EXP = mybir.ActivationFunctionType.Exp
                # PE-critical startup DMAs ride this queue in parallel with
                # SP's: the two y^T chunks of n-group 0, then ey quarter 0
                scalar.dma_start(out=yT_sb[:, 0:4], in_=yT_d[:, 0:4]
                                 ).then_inc(s_ybA[0], 16)
                scalar.dma_start(out=yT_sb[:, 4:8], in_=yT_d[:, 4:8]
                                 ).then_inc(s_ybA[1], 16)
                scalar.dma_start(out=ey_sb[:, 0:NG], in_=ey_d[:, 0:NG]
                                 ).then_inc(s_ey[0], 16)
                scalar.wait_ge(s_x2, 16)
                # group 0 at 512 grain
                for nn in range(NNS):
                    scalar.wait_ge(s_mm, nn + 1)
                    scalar.activation(
                        o_sb[:, 0, nn * NT:(nn + 1) * NT],
                        ps[:, 0, nn * NT:(nn + 1) * NT], EXP,
                        bias=x2_sb[:, 0:1],
                        scale=float(scale2g)).then_inc(s_act, 1)
                for g in range(1, G):
                    ng, m = g // MT, g % MT
                    sl = g % 2
                    osl = g % O_SLOTS
                    if g >= O_SLOTS:
                        scalar.wait_ge(s_dve, g)
                    if g < G - 1:
                        scalar.wait_ge(s_mm, g + 4)
                        scalar.activation(
                            o_sb[:, osl], ps[:, sl], EXP,
                            bias=x2_sb[:, m:m + 1],
                            scale=float(scale2g)).then_inc(s_act, 1)
                    else:
                        for nn in range(NNS):
                            scalar.wait_ge(s_mm, g + 4 + nn)
                            scalar.activation(
                                o_sb[:, osl, nn * NT:(nn + 1) * NT],
                                ps[:, sl, nn * NT:(nn + 1) * NT], EXP,
                                bias=x2_sb[:, m:m + 1],
                                scale=float(scale2g)).then_inc(s_act, 1)
                        # tail DMAs for nn2/nn3 (SP covers nn0/nn1)
                        msl = slice(m * P, (m + 1) * P)
                        for nn in (2, 3):
                            scalar.wait_ge(s_dve, g + 4 + nn)
                            nsl = slice(ng * NG + nn * NT,
                                        ng * NG + (nn + 1) * NT)
                            scalar.dma_start(
                                out=out_d[msl, nsl],
                                in_=o2_sb[:, osl, nn * NT:(nn + 1) * NT]
                            ).then_inc(s_osl[osl], 16)

            @block.vector
            def _(vector):
                vector.memset(scr_sb[:], 0.0).then_inc(s_scr, 1)
                vector.wait_ge(s_ey[0], 16)
                for nn in range(NNS):
                    vector.wait_ge(s_act, nn + 1)
                    vector.tensor_mul(
                        o2_sb[:, 0, nn * NT:(nn + 1) * NT],
                        o_sb[:, 0, nn * NT:(nn + 1) * NT],
                        ey_sb[:, nn * NT:(nn + 1) * NT]).then_inc(s_dve, 1)
                for g in range(1, G):
                    ng, m = g // MT, g % MT
                    osl = g % O_SLOTS
                    gsl = slice(ng * NG, (ng + 1) * NG)
                    if m == 0:  # first group touching ey quarter ng
                        vector.wait_ge(s_ey[ng], 16)
                    if g >= O_SLOTS:
                        vector.wait_ge(s_osl[osl], 16 * (g // O_SLOTS))
                    if g < G - 1:
                        vector.wait_ge(s_act, g + 4)
                        vector.tensor_mul(o2_sb[:, osl], o_sb[:, osl],
                                          ey_sb[:, gsl]).then_inc(s_dve, 1)
                    else:
                        for nn in range(NNS):
                            vector.wait_ge(s_act, g + 4 + nn)
                            nsl = slice(ng * NG + nn * NT,
                                        ng * NG + (nn + 1) * NT)
                            vector.tensor_mul(
                                o2_sb[:, osl, nn * NT:(nn + 1) * NT],
                                o_sb[:, osl, nn * NT:(nn + 1) * NT],
                                ey_sb[:, nsl]).then_inc(s_dve, 1)

        nc.compile()
    return nc


def _pack_xT(xq: np.ndarray) -> np.ndarray:
    """[MC, D] fp8 -> [128, 2*MT, 2, 128]; [p, 2m+kp, s, c] =
    x[m*128 + c, kp*256 + s*128 + p]."""
    a = xq.reshape(MT, P, KP, 2, P)        # [m, c, kp, s, p]
    a = a.transpose(4, 0, 2, 3, 1)         # [p, m, kp, s, c]
    return np.ascontiguousarray(a.reshape(P, 2 * MT, 2, P))


def _pack_yT(yq: np.ndarray) -> np.ndarray:
    """[M, D] fp8 -> [128, 2*NB, 2, NT]; [p, 2nb+kp, s, c] =
    y[nb*512 + c, kp*256 + s*128 + p]."""
    a = yq.reshape(NB, NT, KP, 2, P)       # [nb, c, kp, s, p]
    a = a.transpose(4, 0, 2, 3, 1)         # [p, nb, kp, s, c]
    return np.ascontiguousarray(a.reshape(P, 2 * NB, 2, NT))


def kernel(x: np.ndarray, y: np.ndarray, gamma: np.ndarray) -> np.ndarray:
    from concourse.bass_utils import run_bass_kernel_spmd

    x = np.asarray(x, dtype=np.float32)
    y = np.asarray(y, dtype=np.float32)
    g = float(np.asarray(gamma))

    n, d = x.shape
    m = y.shape[0]
    assert (n, d, m) == (N_FULL, D, M_FULL), (n, d, m)

    key = g
    if key not in _cache:
        _cache.clear()
        _cache[key] = _build_program(2.0 * g)
    nc = _cache[key]

    # host-side prep (O(N*D), ~0.01% of kernel FLOPs)
    fp8 = ml_dtypes.float8_e4m3
    bf16 = ml_dtypes.bfloat16
    yTq = _pack_yT(y.astype(fp8))
    y2 = np.einsum("md,md->m", y, y, dtype=np.float64)
    ey_row = np.exp(-g * y2).astype(bf16)
    eyb = np.ascontiguousarray(np.broadcast_to(ey_row, (P, M_FULL)))
    x2 = np.einsum("nd,nd->n", x, x, dtype=np.float64)

    in_maps = []
    for c in range(N_CORES):
        sl = slice(c * MC, (c + 1) * MC)
        x2_c = np.ascontiguousarray(
            (-g * x2[sl]).astype(np.float32).reshape(MT, P).T)      # [128, MT]
        in_maps.append({"xTq": _pack_xT(x[sl].astype(fp8)), "yTq": yTq,
                        "eyb": eyb, "x2b": x2_c})

    trace = bool(int(os.environ.get("RBF_TRACE", "0")))
    res = run_bass_kernel_spmd(nc, in_maps, core_ids=list(range(N_CORES)),
                               trace=trace)
    global LAST_RESULTS
    LAST_RESULTS = res
    return np.concatenate(
        [r["out"].astype(np.float32) for r in res.results], axis=0)


LAST_RESULTS = None


# revision 20
# speedup vs baseline: 1.5688x; 1.0055x over previous
# RBF Gram matrix kernel for Trainium2 (8 NeuronCores, SPMD).
#
# reference:  G[i, j] = exp(-gamma * ||x_i - y_j||^2)
#                    = exp(2*gamma*(x@y^T)[i,j] - gamma*||x_i||^2) * exp(-gamma*||y_j||^2)
#
# Sharding: row-shard x across 8 cores (1024 rows each), replicate y.
# Each core computes a [1024, 8192] slice of G:
#   PE   : xy = x_c @ y^T     fp8(e4m3) DoubleRow matmuls — K=512 as two
#          256-deep passes, 2 MACs/cell/cycle (~2x bf16 rate), fp32 PSUM
#   ACT  : o = Exp(2g*xy + (-g*||x||^2))  straight from PSUM (bias is the
#          per-partition x-norm vector, scale the 2*gamma immediate)
#   DVE  : o2 = o * exp(-g*||y||^2)   bf16*bf16 at 2x_1P rate
#   DMA  : o2 tile (bf16) -> DRAM; host upcasts to fp32
#
# The steady state is ACT-bound (~2.0us per [128,2048] group vs 1.73us PE),
# so the schedule aims ACT back-to-back from ~11us: fine-grained startup DMAs
# (first n-blocks as 256KB singles), a ~2.6us PE warmup that hands off to
# DMA-paced real matmuls with no >3.4us gap (keeps the HAM clock-gate warm),
# group 0 split at 512-wide grain to fill the ACT pipe early, and a split
# tail whose last DMAs are issued from both SP and ACT queues.
import os

import numpy as np
import ml_dtypes

N_CORES = 8
N_FULL = 8192          # rows of x (and of G)
M_FULL = 8192          # rows of y (cols of G)
D = 512                # feature dim (contraction)
MC = N_FULL // N_CORES # 1024 rows of x per core
P = 128                # SBUF partitions
NT = 512               # matmul moving tile (one fp32 psum bank)
KP = D // (2 * P)      # 2 DoubleRow k-passes (256 contraction each)
MT = MC // P           # 8 m-tiles per core
NB = M_FULL // NT      # 16 n-blocks of 512
NG = 2048              # psum slot width: 4 banks
NGR = M_FULL // NG     # 4 n-groups
NNS = NG // NT         # 4 n-blocks per group

_cache = {}


def _build_program(scale2g: float):
    """Raw-Bass build: explicit per-engine programs + hand-rolled semaphores."""
    from contextlib import ExitStack, contextmanager

    import concourse.bass as bass
    import concourse.mybir as mybir
    from concourse import bacc

    class _NoBarrierBlock(bass.BassBlock):
        """BassBlock whose exit emits per-engine drains but no all-engine
        barrier; cross-engine ordering is fully covered by our semaphores."""

        def __exit__(self, exc_type, exc_val, exc_tb):
            if exc_type is not None:
                return
            for engine, last_body in self.last_body.items():
                with self.bass.body(last_body, parent=self.bass.cur_bb,
                                    allow_existing_parent=True):
                    engine.br(self.end_bb)
            self.bass.switch_bb(self.end_bb)
            gpsimd_type = self.bass.gpsimd.engine
            for eng_type, eng in self.bass.engines.items():
                if eng_type == gpsimd_type:
                    continue
                dr = mybir.InstDrain(
                    name=self.bass.get_next_instruction_name(),
                    ins=[], outs=[], bass_is_fusable=False)
                dr.engine = eng_type
                eng.add_instruction(dr)

    @contextmanager
    def _no_barrier_block(nc):
        assert nc.cur_block is None
        blk = _NoBarrierBlock(nc, f"block_{nc.next_id()}")
        nc.cur_block = blk
        try:
            with blk:
                yield blk
        finally:
            nc.cur_block = None

    DR = mybir.MatmulPerfMode.DoubleRow
    G = NGR * MT           # 32 pipeline groups of [128, 2048]
    O_SLOTS = 4
    NWARM = 44             # PE busy until the first y^T chunk lands, so the
                           # HAM clock-gate is warm when real matmuls start

    nc = bacc.Bacc("TRN2", target_bir_lowering=False, debug=False,
                   num_devices=N_CORES)

    # exact SBUF images (see kernel() for the host-side permutes)
    xT_d = nc.dram_tensor("xTq", [P, 2 * MT, 2, P], mybir.dt.float8e4,
                          kind="ExternalInput").ap()
    yT_d = nc.dram_tensor("yTq", [P, 2 * NB, 2, NT], mybir.dt.float8e4,
                          kind="ExternalInput").ap()
    ey_d = nc.dram_tensor("eyb", [P, M_FULL], mybir.dt.bfloat16,
                          kind="ExternalInput").ap()
    x2_d = nc.dram_tensor("x2b", [P, MT], mybir.dt.float32,
                          kind="ExternalInput").ap()
    out_d = nc.dram_tensor("out", [MC, M_FULL], mybir.dt.bfloat16,
                           kind="ExternalOutput").ap()

    with ExitStack() as ctx:
        ec = ctx.enter_context
        xT_sb = ec(nc.sbuf_tensor([P, 2 * MT, 2, P], mybir.dt.float8e4))
        yT_sb = ec(nc.sbuf_tensor([P, 2 * NB, 2, NT], mybir.dt.float8e4))
        ey_sb = ec(nc.sbuf_tensor([P, M_FULL], mybir.dt.bfloat16))
        x2_sb = ec(nc.sbuf_tensor([P, MT], mybir.dt.float32))
        scr_sb = ec(nc.sbuf_tensor([P, 2 * P], mybir.dt.bfloat16))
        o_sb = ec(nc.sbuf_tensor([P, O_SLOTS, NG], mybir.dt.bfloat16))
        o2_sb = ec(nc.sbuf_tensor([P, O_SLOTS, NG], mybir.dt.bfloat16))
        # 2 super-slots x 2048 fp32 = all 8 PSUM banks; PE cycles them as 4
        # logical 1024-wide sub-slots so the ACT->PE recycle semaphore
        # round-trip is hidden by ring slack
        ps = ec(nc.psum_tensor([P, 2, NG], mybir.dt.float32))

        s_scr = ec(nc.semaphore(name="s_scr"))
        s_xm0 = ec(nc.semaphore(name="s_xm0"))
        s_xr = ec(nc.semaphore(name="s_xr"))
        s_ybA = [ec(nc.semaphore(name=f"s_ybA{i}")) for i in range(2)]
        s_ybB = [ec(nc.semaphore(name=f"s_ybB{i}")) for i in range(3)]
        s_x2 = ec(nc.semaphore(name="s_x2"))
        s_ey = [ec(nc.semaphore(name=f"s_ey{i}")) for i in range(NGR)]
        s_mm = ec(nc.semaphore(name="s_mm"))
        s_act = ec(nc.semaphore(name="s_act"))
        s_dve = ec(nc.semaphore(name="s_dve"))
        s_osl = [ec(nc.semaphore(name=f"s_osl{i}")) for i in range(O_SLOTS)]

        # completion counts:
        #   s_mm  counts 1024-wide PE sub-groups (h); the final two halves
        #         inc at 512 grain: after h: h+1 (h<=61), tail 63..66
        #   s_act/s_dve count 2048-wide ACT/DVE groups (j): after j: j+1
        #         (j<=30); the final group j=31 incs at 512 grain: 32..35
        with _no_barrier_block(nc) as block:

            @block.sync
            def _(sync):
                # startup set, in critical-path order (the two 512KB y^T
                # chunks of n-group 0 and the first exp(-g*y^2) quarter ride
                # the ACT queue in parallel — see block.scalar)
                sync.dma_start(out=xT_sb[:, 0:2], in_=xT_d[:, 0:2]
                               ).then_inc(s_xm0, 16)
                sync.dma_start(out=xT_sb[:, 2:], in_=xT_d[:, 2:]
                               ).then_inc(s_xr, 16)
                sync.dma_start(out=x2_sb[:], in_=x2_d).then_inc(s_x2, 16)
                sync.dma_start(out=yT_sb[:, 8:16], in_=yT_d[:, 8:16]
                               ).then_inc(s_ybB[0], 16)
                sync.dma_start(out=ey_sb[:, NG:2 * NG],
                               in_=ey_d[:, NG:2 * NG]).then_inc(s_ey[1], 16)
                sync.dma_start(out=yT_sb[:, 16:24], in_=yT_d[:, 16:24]
                               ).then_inc(s_ybB[1], 16)
                sync.dma_start(out=ey_sb[:, 2 * NG:3 * NG],
                               in_=ey_d[:, 2 * NG:3 * NG]).then_inc(s_ey[2], 16)
                sync.dma_start(out=yT_sb[:, 24:32], in_=yT_d[:, 24:32]
                               ).then_inc(s_ybB[2], 16)
                sync.dma_start(out=ey_sb[:, 3 * NG:],
                               in_=ey_d[:, 3 * NG:]).then_inc(s_ey[3], 16)
                for j in range(G):
                    ng, m = j // MT, j % MT
                    sl = j % O_SLOTS
                    msl = slice(m * P, (m + 1) * P)
                    if j < G - 1:
                        sync.wait_ge(s_dve, j + 1)
                        sync.dma_start(
                            out=out_d[msl, ng * NG:(ng + 1) * NG],
                            in_=o2_sb[:, sl]).then_inc(s_osl[sl], 16)
                    else:
                        # tail: nn0/nn1 from here, nn2/nn3 from the ACT queue
                        for nn in range(2):
                            sync.wait_ge(s_dve, 32 + nn)
                            nsl = slice(ng * NG + nn * NT,
                                        ng * NG + (nn + 1) * NT)
                            sync.dma_start(
                                out=out_d[msl, nsl],
                                in_=o2_sb[:, sl, nn * NT:(nn + 1) * NT]
                            ).then_inc(s_osl[sl], 16)
                # the end-of-block DRAIN quiesces the DGE queues, so no
                # explicit waits on the final transfer completions here

            @block.tensor
            def _(tensor):
                tensor.wait_ge(s_scr, 1)
                for _ in range(NWARM):
                    tensor.matmul(ps[:, 0, 0:P], lhsT=scr_sb[:, P:2 * P],
                                  rhs=scr_sb[:, 0:P], start=True, stop=True)
                tensor.wait_ge(s_xm0, 16)
                for g in range(G):
                    ng, m = g // MT, g % MT
                    sl = g % 2
                    if g == 0:
                        tensor.wait_ge(s_ybA[0], 16)
                    if g == 1:
                        tensor.wait_ge(s_xr, 16)
                    if m == 0 and ng >= 1:
                        tensor.wait_ge(s_ybB[ng - 1], 16)
                    if g >= 2:
                        tensor.wait_ge(s_act, g - 1)
                    if g == G - 1:
                        # final group: kp-inner per 512 so the drain chain
                        # can start at fine grain
                        for nn in range(NNS):
                            for kp in range(KP):
                                inst = tensor.matmul(
                                    ps[:, sl, nn * NT:(nn + 1) * NT],
                                    lhsT=xT_sb[:, 2 * m + kp],
                                    rhs=yT_sb[:, (NNS * ng + nn) * 2 + kp],
                                    start=(kp == 0),
                                    stop=(kp == KP - 1),
                                    perf_mode=DR,
                                )
                            inst.then_inc(s_mm, 1)
                    else:
                        for kp in range(KP):
                            for nn in range(NNS):
                                if g == 0 and kp == 0 and nn == 2:
                                    tensor.wait_ge(s_ybA[1], 16)
                                inst = tensor.matmul(
                                    ps[:, sl, nn * NT:(nn + 1) * NT],
                                    lhsT=xT_sb[:, 2 * m + kp],
                                    rhs=yT_sb[:, (NNS * ng + nn) * 2 + kp],
                                    start=(kp == 0),
                                    stop=(kp == KP - 1),
                                    perf_mode=DR,
                                )
                        inst.then_inc(s_mm, 1)

            @block.scalar
            def _(scalar):
                EXP = mybir.ActivationFunctionType.Exp
                # PE-critical startup DMAs ride this queue in parallel with
                # SP's: the two y^T chunks of n-group 0, then ey quarter 0
                scalar.dma_start(out=yT_sb[:, 0:4], in_=yT_d[:, 0:4]
                                 ).then_inc(s_ybA[0], 16)
                scalar.dma_start(out=yT_sb[:, 4:8], in_=yT_d[:, 4:8]
                                 ).then_inc(s_ybA[1], 16)
                scalar.dma_start(out=ey_sb[:, 0:NG], in_=ey_d[:, 0:NG]
                                 ).then_inc(s_ey[0], 16)
                scalar.wait_ge(s_x2, 16)
                for g in range(G):
                    ng, m = g // MT, g % MT
                    sl = g % 2
                    osl = g % O_SLOTS
                    if g >= O_SLOTS:
                        scalar.wait_ge(s_dve, g - 3)
                    if g < G - 1:
                        scalar.wait_ge(s_mm, g + 1)
                        scalar.activation(
                            o_sb[:, osl], ps[:, sl], EXP,
                            bias=x2_sb[:, m:m + 1],
                            scale=float(scale2g)).then_inc(s_act, 1)
                    else:
                        for nn in range(NNS):
                            scalar.wait_ge(s_mm, 32 + nn)
                            scalar.activation(
                                o_sb[:, osl, nn * NT:(nn + 1) * NT],
                                ps[:, sl, nn * NT:(nn + 1) * NT], EXP,
                                bias=x2_sb[:, m:m + 1],
                                scale=float(scale2g)).then_inc(s_act, 1)
                        # tail DMAs for nn2/nn3 (SP covers nn0/nn1)
                        msl = slice(m * P, (m + 1) * P)
                        for nn in (2, 3):
                            scalar.wait_ge(s_dve, 32 + nn)
                            nsl = slice(ng * NG + nn * NT,
                                        ng * NG + (nn + 1) * NT)
                            scalar.dma_start(
                                out=out_d[msl, nsl],
                                in_=o2_sb[:, osl, nn * NT:(nn + 1) * NT]
                            ).then_inc(s_osl[osl], 16)

            @block.vector
            def _(vector):
                vector.memset(scr_sb[:], 0.0).then_inc(s_scr, 1)
                for g in range(G):
                    ng, m = g // MT, g % MT
                    osl = g % O_SLOTS
                    gsl = slice(ng * NG, (ng + 1) * NG)
                    if m == 0:  # first group touching ey quarter ng
                        vector.wait_ge(s_ey[ng], 16)
                    if g >= O_SLOTS:
                        vector.wait_ge(s_osl[osl], 16 * (g // O_SLOTS))
                    if g < G - 1:
                        vector.wait_ge(s_act, g + 1)
                        vector.tensor_mul(o2_sb[:, osl], o_sb[:, osl],
                                          ey_sb[:, gsl]).then_inc(s_dve, 1)
                    else:
                        for nn in range(NNS):
                            vector.wait_ge(s_act, 32 + nn)
                            nsl = slice(ng * NG + nn * NT,
                                        ng * NG + (nn + 1) * NT)
                            vector.tensor_mul(
                                o2_sb[:, osl, nn * NT:(nn + 1) * NT],
                                o_sb[:, osl, nn * NT:(nn + 1) * NT],
                                ey_sb[:, nsl]).then_inc(s_dve, 1)

        nc.compile()
    return nc


def _pack_xT(xq: np.ndarray) -> np.ndarray:
    """[MC, D] fp8 -> [128, 2*MT, 2, 128]; [p, 2m+kp, s, c] =
    x[m*128 + c, kp*256 + s*128 + p]."""
    a = xq.reshape(MT, P, KP, 2, P)        # [m, c, kp, s, p]
    a = a.transpose(4, 0, 2, 3, 1)         # [p, m, kp, s, c]
    return np.ascontiguousarray(a.reshape(P, 2 * MT, 2, P))


def _pack_yT(yq: np.ndarray) -> np.ndarray:
    """[M, D] fp8 -> [128, 2*NB, 2, NT]; [p, 2nb+kp, s, c] =
    y[nb*512 + c, kp*256 + s*128 + p]."""
    a = yq.reshape(NB, NT, KP, 2, P)       # [nb, c, kp, s, p]
    a = a.transpose(4, 0, 2, 3, 1)         # [p, nb, kp, s, c]
    return np.ascontiguousarray(a.reshape(P, 2 * NB, 2, NT))


def kernel(x: np.ndarray, y: np.ndarray, gamma: np.ndarray) -> np.ndarray:
    from concourse.bass_utils import run_bass_kernel_spmd

    x = np.asarray(x, dtype=np.float32)
    y = np.asarray(y, dtype=np.float32)
    g = float(np.asarray(gamma))

    n, d = x.shape
    m = y.shape[0]
    assert (n, d, m) == (N_FULL, D, M_FULL), (n, d, m)

    key = g
    if key not in _cache:
        _cache.clear()
        _cache[key] = _build_program(2.0 * g)
    nc = _cache[key]

    # host-side prep (O(N*D), ~0.01% of kernel FLOPs)
    fp8 = ml_dtypes.float8_e4m3
    bf16 = ml_dtypes.bfloat16
    yTq = _pack_yT(y.astype(fp8))
    y2 = np.einsum("md,md->m", y, y, dtype=np.float64)
    ey_row = np.exp(-g * y2).astype(bf16)
    eyb = np.ascontiguousarray(np.broadcast_to(ey_row, (P, M_FULL)))
    x2 = np.einsum("nd,nd->n", x, x, dtype=np.float64)

    in_maps = []
    for c in range(N_CORES):
        sl = slice(c * MC, (c + 1) * MC)
        x2_c = np.ascontiguousarray(
            (-g * x2[sl]).astype(np.float32).reshape(MT, P).T)      # [128, MT]
        in_maps.append({"xTq": _pack_xT(x[sl].astype(fp8)), "yTq": yTq,
                        "eyb": eyb, "x2b": x2_c})

    trace = bool(int(os.environ.get("RBF_TRACE", "0")))
    res = run_bass_kernel_spmd(nc, in_maps, core_ids=list(range(N_CORES)),
                               trace=trace)
    global LAST_RESULTS
    LAST_RESULTS = res
    return np.concatenate(
        [r["out"].astype(np.float32) for r in res.results], axis=0)


LAST_RESULTS = None
